# revision 1
# baseline (speedup 1.0000x reference)
"""Akima spline interpolation kernel for Trainium2 (8 NeuronCores, data parallel).

Strategy:
  - Host precomputes Akima node slopes and per-interval cubic coefficients
    from the tiny 256-knot `value` parameter, splits off the exact linear
    part (out = x + correction(255*x) for in-range x), Taylor-shifts the
    correction polynomial to the interval midpoint, and quantizes the four
    correction coefficients to int8 with one shared symmetric scale,
    packed into one uint32 word per interval (256-word table).
  - Device (NKI kernel per core): t = 255*x - 0.5 (clamped); floor via the
    1.5*2^23 magic-constant trick on the Scalar engine (the fractional
    residue vv in [-0.5, 0.5] and the gather index derive from the same
    rounding, so they are consistent by construction); one
    nl.gather_flattened on GPSIMD fetches the packed word per element; the
    cubic is evaluated by Horner directly on the int8 lanes (mixed-dtype
    tensor_tensor ops); the final scalar_tensor_tensor applies the shared
    scale and adds x back.
  - Sharding: pure data parallel on the leading dim (4 of 32 planes per
    core); the 1KB table is replicated to all partitions of every core.
"""
import base64
import json
import sys

import numpy as np

if "/opt/trn_rl_repo" not in sys.path:
    sys.path.insert(0, "/opt/trn_rl_repo")

NODES = 256
NI = NODES - 1
N_CORES = 8
ROWS = 128
COLS = 4 * 1024 * 1024 // ROWS  # per-core shard [128, 32768]
F_TILE = 2048
MAGIC = float(np.float32(1.5 * 2.0 ** 23))

# ----------------------------------------------------------------------------
# Host-side table construction
# ----------------------------------------------------------------------------


def _akima_slopes_f64(value):
    h = 1.0 / (NODES - 1)
    v = value.astype(np.float64)
    m = (v[1:] - v[:-1]) / h
    m_m1 = 2.0 * m[0] - m[1]
    m_m2 = 2.0 * m_m1 - m[0]
    m_p1 = 2.0 * m[-1] - m[-2]
    m_p2 = 2.0 * m_p1 - m[-1]
    me = np.concatenate([[m_m2, m_m1], m, [m_p1, m_p2]])
    w1 = np.abs(me[3:] - me[2:-1])
    w2 = np.abs(me[1:-2] - me[:-3])
    mi_1 = me[1:-2]
    mi = me[2:-1]
    denom = w1 + w2
    safe = np.where(denom > 0, denom, 1.0)
    return np.where(denom > 0, (w1 * mi_1 + w2 * mi) / safe, 0.5 * (mi_1 + mi))


def _build_table(value):
    h = 1.0 / (NODES - 1)
    s = _akima_slopes_f64(value)
    v = value.astype(np.float64)
    v0, v1 = v[:-1], v[1:]
    hs0, hs1 = h * s[:-1], h * s[1:]
    c0 = v0
    c1 = hs0
    c2 = -3 * v0 + 3 * v1 - 2 * hs0 - hs1
    c3 = 2 * v0 - 2 * v1 + hs0 + hs1
    i_arr = np.arange(NI)
    # subtract the exact linear part: x = (i + u) * h
    d0 = c0 - i_arr * h
    d1 = c1 - h
    d2, d3 = c2, c3
    # Taylor shift to the interval midpoint: u = vv + 1/2
    e0 = d0 + 0.5 * d1 + 0.25 * d2 + 0.125 * d3
    e1 = d1 + d2 + 0.75 * d3
    e2 = d2 + 1.5 * d3
    e3 = d3
    es = [e0, e1, e2, e3]
    S = max(max(abs(e.min()), abs(e.max())) for e in es) / 127.0
    qs = [np.clip(np.round(e / S), -127, 127).astype(np.int64) for e in es]
    word = ((qs[0] & 0xFF) | ((qs[1] & 0xFF) << 8)
            | ((qs[2] & 0xFF) << 16) | ((qs[3] & 0xFF) << 24)).astype(np.uint32)
    word = np.concatenate([word, word[-1:]])  # pad to 256 entries
    return word, float(S)


# ----------------------------------------------------------------------------
# NKI kernel
# ----------------------------------------------------------------------------


def _make_nki_kernel(S):
    import neuronxcc.nki.language as nl
    import neuronxcc.nki.isa as nisa

    n_tiles = COLS // F_TILE

    def akima_kernel(inputs):
        x, table = inputs[0], inputs[1]
        out = nl.ndarray(shape=[ROWS, COLS], dtype=nl.float32, buffer=nl.shared_hbm)
        tab_sb = nl.load(table)
        i_p = nl.arange(ROWS)[:, None]
        i_f = nl.arange(F_TILE)[None, :]
        magic_bias = nisa.memset((ROWS, 1), MAGIC, nl.float32)
        neg_magic_bias = nisa.memset((ROWS, 1), -MAGIC, nl.float32)

        # Explicit ping-pong SBUF buffers: without them the allocator's
        # address reuse creates WAR hazards that serialize consecutive tiles.
        def mkbufs():
            return dict(
                ts1=nl.ndarray(shape=[ROWS, F_TILE], dtype=nl.float32, buffer=nl.sbuf),
                rbig=nl.ndarray(shape=[ROWS, F_TILE], dtype=nl.float32, buffer=nl.sbuf),
                idxf=nl.ndarray(shape=[ROWS, F_TILE], dtype=nl.float32, buffer=nl.sbuf),
                idx=nl.ndarray(shape=[ROWS, F_TILE], dtype=nl.uint32, buffer=nl.sbuf),
                w=nl.ndarray(shape=[ROWS, F_TILE], dtype=nl.uint32, buffer=nl.sbuf),
                vv=nl.ndarray(shape=[ROWS, F_TILE], dtype=nl.float32, buffer=nl.sbuf),
            )

        bufs = [mkbufs(), mkbufs()]

        for t in range(n_tiles):
            B = bufs[t % 2]
            sl = slice(t * F_TILE, (t + 1) * F_TILE)
            x_sb = nl.load(x[:, sl])
            B['ts1'][i_p, i_f] = nisa.tensor_scalar(
                x_sb, np.multiply, 255.0, op1=np.subtract, operand1=0.5)
            B['rbig'][i_p, i_f] = nisa.activation(
                np.copy, B['ts1'][i_p, i_f], bias=magic_bias)
            B['idxf'][i_p, i_f] = nisa.activation(
                np.copy, B['rbig'][i_p, i_f], bias=neg_magic_bias)
            B['idx'][i_p, i_f] = nisa.activation(
                np.copy, B['idxf'][i_p, i_f], dtype=nl.uint32)
            B['w'][i_p, i_f] = nl.gather_flattened(
                data=tab_sb, indices=B['idx'][i_p, i_f])
            B['vv'][i_p, i_f] = nisa.tensor_tensor(
                B['ts1'][i_p, i_f], B['idxf'][i_p, i_f], np.subtract)
            w8 = B['w'].view(nl.int8)
            vvr = B['vv'][i_p, i_f]
            m = nisa.tensor_tensor(w8[i_p, i_f * 4 + 3], vvr, np.multiply,
                                   dtype=nl.float32, engine=nisa.vector_engine)
            m = nisa.tensor_tensor(m, w8[i_p, i_f * 4 + 2], np.add,
                                   dtype=nl.float32)
            m = nisa.tensor_tensor(m, vvr, np.multiply)
            m = nisa.tensor_tensor(m, w8[i_p, i_f * 4 + 1], np.add,
                                   dtype=nl.float32)
            m = nisa.tensor_tensor(m, vvr, np.multiply)
            m = nisa.tensor_tensor(m, w8[i_p, i_f * 4 + 0], np.add,
                                   dtype=nl.float32)
            r = nisa.scalar_tensor_tensor(data=m, op0=np.multiply, operand0=S,
                                          op1=np.add, operand1=x_sb)
            nl.store(out[:, sl], r)
        return [out]

    return akima_kernel


# ----------------------------------------------------------------------------
# jax integration (AwsNeuronCustomNativeKernel custom call, SPMD over 8 cores)
# ----------------------------------------------------------------------------

_EXEC_CACHE = {}


def _build_executor(S):
    if S in _EXEC_CACHE:
        return _EXEC_CACHE[S]

    import jax
    from jax.interpreters import mlir
    from jax._src.interpreters.mlir import custom_call as _mlir_custom_call
    from jax.sharding import Mesh, PartitionSpec
    from jax.experimental.shard_map import shard_map
    from concourse.nki import raw_nki
    from concourse.bass2jax import install_neuronx_cc_hook

    install_neuronx_cc_hook()

    nki_func = _make_nki_kernel(S)

    prim = jax.extend.core.Primitive(f"akima_exec_{len(_EXEC_CACHE)}")
    prim.multiple_results = True

    @prim.def_abstract_eval
    def _abs(*_, **__):
        return (jax.core.ShapedArray((ROWS, COLS), np.float32),)

    def _layouts(shapes):
        return [list(reversed(range(len(s)))) for s in shapes]

    def _lowering(ctx, *in_nodes):
        from neuronxcc.starfish.penguin.ir.NativeKernel import KERNEL_VERSION

        result_types = [mlir.aval_to_ir_type(a) for a in ctx.avals_out]
        code = raw_nki(nki_func)(list(ctx.avals_in))
        config = {
            "kernel_version": KERNEL_VERSION,
            "func_literal": code.serialize_ir_string("akima_kernel_ir"),
            "grid": [],
            "func_name": "akima_kernel",
            "has_collectives": False,
            "mac_count": 0,
            "tiled": False,
        }
        dumped = base64.b64encode(json.dumps(config).encode()).decode()
        return _mlir_custom_call(
            "AwsNeuronCustomNativeKernel",
            operands=list(in_nodes),
            result_types=result_types,
            operand_layouts=_layouts(a.shape for a in ctx.avals_in),
            result_layouts=_layouts(a.shape for a in ctx.avals_out),
            backend_config=dumped,
        ).results

    mlir.register_lowering(prim, _lowering, platform="neuron")

    devices = jax.devices()[:N_CORES]
    mesh = Mesh(np.asarray(devices), ("core",))

    def _body(x_shard, tab_shard):
        return prim.bind(x_shard, tab_shard)[0]

    sharded = jax.jit(shard_map(
        _body, mesh=mesh,
        in_specs=(PartitionSpec("core"), PartitionSpec("core")),
        out_specs=PartitionSpec("core"),
        check_rep=False,
    ))

    _EXEC_CACHE[S] = sharded
    return sharded


# ----------------------------------------------------------------------------
# Public entry point
# ----------------------------------------------------------------------------


def kernel(input: np.ndarray, value: np.ndarray) -> np.ndarray:
    input = np.ascontiguousarray(np.asarray(input, dtype=np.float32))
    value = np.asarray(value, dtype=np.float32)
    assert input.shape == (32, 1024, 1024), input.shape

    word, S = _build_table(value)
    table = np.broadcast_to(word, (ROWS, NODES)).copy()

    sharded = _build_executor(S)

    # shard on the leading dim: core i gets planes [4i, 4i+4)
    x_global = input.reshape(N_CORES * ROWS, COLS)
    tab_global = np.tile(table, (N_CORES, 1))

    out = sharded(x_global, tab_global)
    return np.asarray(out).reshape(32, 1024, 1024)


if __name__ == "__main__":
    inp = np.load("cache/input.npy")
    val = np.load("cache/value.npy")
    out = kernel(input=inp, value=val)
    exp = np.load("cache/expected.npy")
    err = out.astype(np.float64) - exp.astype(np.float64)
    print("rel_l2:", np.linalg.norm(err) / np.linalg.norm(exp))

